# revision 30
# baseline (speedup 1.0000x reference)
"""Multi-head self-attention (B=4,S=2048,D=1024,H=16,DH=64, causal) on 8 trn2 cores.

Sharding: core c -> batch b=c//2, head-group g=c%2 (8 heads each).
Per-core pipeline:
- QKV projections in bf16 (PE), streamed per 512-wide s block and software-
  pipelined against attention of the previous q-block.
- Scores in bf16. (fp8 DoubleRow was tried and measured SLOWER: fp8 matmuls
  pay ~150-210ns PE transition penalties that eat the 2x column-rate gain.)
- Causal mask added in PSUM by a small [-30*I] @ [c<p] matmul on the 128-wide
  diagonal strip; above-diagonal columns of diagonal tiles are skipped.
- exp on ACT engine batched over k-tile pairs ([128,1024] two-bank PSUM reads).
- attn@V in bf16 with ones-column denominator; renorm via
  reciprocal_approx_fast (DVE) + partition_broadcast (gpsimd) + mult (DVE).
- bv and bp folded host-side into the output-projection bias.
- Two heads' attention streams interleaved to widen the exp-shadow window.
Host sums the two head-group partial outputs per batch.

K-projection quirk (reference views k as (B,S,DH,H)): head h uses Wk rows
[dh*16+h for dh in range(64)] -- handled by host-side row gather.
"""
import numpy as np
import ml_dtypes

import concourse.mybir as mybir
import concourse.tile as tile
from concourse import bacc
from concourse.bass_utils import run_bass_kernel_spmd

F32 = mybir.dt.float32
BF16 = mybir.dt.bfloat16
AF = mybir.ActivationFunctionType
ALU = mybir.AluOpType

B, S, D, H, DH = 4, 2048, 1024, 16, 64
FG = 512          # features per head-group (8 heads * 64)
N_CORES = 8
SCALE = 0.125     # 1/sqrt(64)

_NC = None


def _build():
    nc = bacc.Bacc("TRN2", target_bir_lowering=False, debug=False,
                   num_devices=N_CORES, enable_asserts=False)
    xbT_d = nc.dram_tensor("xbT", [D, S], BF16, kind="ExternalInput").ap()
    wqT_d = nc.dram_tensor("wqT", [D, FG], BF16, kind="ExternalInput").ap()
    wkT_d = nc.dram_tensor("wkT", [D, FG], BF16, kind="ExternalInput").ap()
    wvT_d = nc.dram_tensor("wvT", [D, FG], BF16, kind="ExternalInput").ap()
    wpT_d = nc.dram_tensor("wpT", [FG, D], BF16, kind="ExternalInput").ap()
    bqs_d = nc.dram_tensor("bqs", [128, 4], F32, kind="ExternalInput").ap()
    bks_d = nc.dram_tensor("bks", [128, 4], F32, kind="ExternalInput").ap()
    bps_d = nc.dram_tensor("bps", [128, 8], F32, kind="ExternalInput").ap()
    negI_d = nc.dram_tensor("negI", [128, 128], BF16, kind="ExternalInput").ap()
    ltB_d = nc.dram_tensor("ltB", [128, 128], BF16, kind="ExternalInput").ap()
    out_d = nc.dram_tensor("outT", [D, S], F32, kind="ExternalOutput").ap()

    with tile.TileContext(nc) as tc:
        with tc.tile_pool(name="persist", bufs=1) as pp, \
             tc.tile_pool(name="xin", bufs=2) as xp, \
             tc.tile_pool(name="etile", bufs=4) as ep, \
             tc.tile_pool(name="small", bufs=4) as sp, \
             tc.tile_pool(name="outtile", bufs=3) as op, \
             tc.tile_pool(name="pspair", bufs=3, space="PSUM") as ps_pair, \
             tc.tile_pool(name="psot", bufs=2, space="PSUM") as ps_ot:

            # ---- persistent SBUF tensors ----
            wq = pp.tile([128, 8, FG], BF16)   # [dp, do, f]  (pre-scaled 1/8)
            wk = pp.tile([128, 8, FG], BF16)
            wv = pp.tile([128, 8, FG], BF16)
            wp = pp.tile([128, 4, D], BF16)    # [cp, co, j]
            qt = pp.tile([128, 4, S], BF16)    # [fp, fo, s]
            kt = pp.tile([128, 4, S], BF16)
            va = pp.tile([128, 16, 8, DH + 1], BF16)  # [skp, sko, h, dh|1]
            on_ = pp.tile([128, 4, S], BF16)   # renormed out^T  [cp, co, s]
            negI = pp.tile([128, 128], BF16)
            ltB = pp.tile([128, 128], BF16)
            bqs = pp.tile([128, 4], F32)
            bks = pp.tile([128, 4], F32)
            bps = pp.tile([128, 8], F32)

            nc.gpsimd.dma_start(wq[:], wqT_d.rearrange("(do dp) f -> dp do f", dp=128))
            nc.gpsimd.dma_start(wk[:], wkT_d.rearrange("(do dp) f -> dp do f", dp=128))
            nc.gpsimd.dma_start(wv[:], wvT_d.rearrange("(do dp) f -> dp do f", dp=128))
            nc.gpsimd.dma_start(wp[:], wpT_d.rearrange("(co cp) j -> cp co j", cp=128))
            nc.sync.dma_start(negI[:], negI_d[:])
            nc.sync.dma_start(ltB[:], ltB_d[:])
            nc.sync.dma_start(bqs[:], bqs_d[:])
            nc.sync.dma_start(bks[:], bks_d[:])
            nc.sync.dma_start(bps[:], bps_d[:])
            nc.vector.memset(va[:, :, :, DH:DH + 1], 1.0)

            xbT_r = xbT_d.rearrange("(do dp) s -> dp do s", dp=128)

            xtiles = {}

            def emit_xdma(sb):
                xb = xp.tile([128, 8, 512], BF16)
                # first block on the sync queue so it overlaps the weight DMAs
                eng = nc.sync if sb == 0 else nc.gpsimd
                eng.dma_start(xb[:], xbT_r[:, :, sb * 512:(sb + 1) * 512])
                xtiles[sb] = xb

            # ---- phase B: QKV projections for one 512-wide s block ----
            # group index: 0-3 Q(ft), 4-7 K(ft), 8-11 V(st)
            def emit_b_group(sb, gi):
                xb = xtiles[sb]
                ssl = slice(sb * 512, (sb + 1) * 512)
                if gi < 8:  # Q or K projection, output [f=128, s=512]
                    ft = gi % 4
                    w_sb = wq if gi < 4 else wk
                    pst = ps_pair.tile([128, 1024], F32, space="PSUM",
                                       tag="pair")
                    ps = pst[:, 0:512]
                    for do in range(8):
                        nc.tensor.matmul(
                            ps, w_sb[:, do, ft * 128:(ft + 1) * 128],
                            xb[:, do, :], start=(do == 0), stop=(do == 7))
                    dst, bias = (qt, bqs) if gi < 4 else (kt, bks)
                    nc.vector.tensor_scalar_add(
                        dst[:, ft, ssl], ps, bias[:, ft:ft + 1])
                else:       # V projection, output [s=128, f=512]
                    st = gi - 8
                    pst = ps_pair.tile([128, 1024], F32, space="PSUM",
                                       tag="pair")
                    ps = pst[:, 0:512]
                    for do in range(8):
                        nc.tensor.matmul(
                            ps, xtiles[sb][:, do, st * 128:(st + 1) * 128],
                            wv[:, do, :], start=(do == 0), stop=(do == 7))
                    nc.vector.tensor_copy(
                        va[:, sb * 4 + st, :, :DH],
                        ps.rearrange("p (h d) -> p h d", h=8))

            # ---- bf16 score matmul for head h, k-tile t, q-block qb ----
            def mm_score(out_ap, h, t, qb, c0, start, stop):
                g2, j = h % 2, h // 2
                p0 = 64 * g2
                lhsT = kt[p0:p0 + 64, j, 128 * t:128 * t + 128]
                rhs = qt[p0:p0 + 64, j, qb * 512 + c0:(qb + 1) * 512]
                nc.tensor.matmul(out_ap, lhsT, rhs, start=start, stop=stop)

            # ---- attention for (q-block qb, head h), generator-staged so
            # two heads' streams can be interleaved (widens the window for
            # hiding exp latency behind the other head's matmuls) ----
            def emit_av(ot, qb, h, et, u, nt):
                for half in range(2):
                    t = 2 * u + half
                    m = t - 4 * qb
                    c0 = 0 if m < 0 else 128 * m
                    hsl = 512 * half
                    nc.tensor.matmul(
                        ot[0:DH + 1, c0:512], va[:, t, h, :],
                        et[:, hsl + c0:hsl + 512],
                        start=(t == 0), stop=(t == nt - 1),
                        skip_group_check=True)

            def gen_c(qb, h):
                nt = 4 * qb + 4
                qsl = slice(qb * 512, (qb + 1) * 512)
                ot = ps_ot.tile([DH + 1, 512], F32, space="PSUM", tag="ot")
                prev = None  # (et, u) whose attn@V is deferred one round
                for u in range(nt // 2):
                    pt = ps_pair.tile([128, 1024], F32, space="PSUM", tag="pair")
                    et = ep.tile([128, 1024], BF16, tag="e")
                    for half in range(2):
                        t = 2 * u + half
                        m = t - 4 * qb
                        hsl = 512 * half
                        if m < 0:  # full tile
                            mm_score(pt[:, hsl:hsl + 512], h, t, qb, 0,
                                     True, True)
                        else:      # diagonal tile: trim cols, add mask strip
                            c0 = 128 * m
                            mm_score(pt[:, hsl + c0:hsl + 512], h, t, qb, c0,
                                     True, False)
                            nc.tensor.matmul(
                                pt[:, hsl + c0:hsl + c0 + 128], negI[:],
                                ltB[:], start=False, stop=True,
                                skip_group_check=True)
                    yield
                    # exp (ACT), batched over the pair when both halves full
                    m0 = 2 * u - 4 * qb
                    if m0 < 0:
                        nc.scalar.activation(et[:], pt[:], AF.Exp)
                    else:
                        c0a, c0b = 128 * m0, 128 * (m0 + 1)
                        nc.scalar.activation(
                            et[:, c0a:512], pt[:, c0a:512], AF.Exp)
                        nc.scalar.activation(
                            et[:, 512 + c0b:1024], pt[:, 512 + c0b:1024],
                            AF.Exp)
                    if prev is not None:
                        emit_av(ot, qb, h, prev[0], prev[1], nt)
                    prev = (et, u)
                    yield
                emit_av(ot, qb, h, prev[0], prev[1], nt)
                # softmax renorm: divide by ones-column row of ot
                dn = sp.tile([1, 512], F32, tag="dn")
                nc.vector.tensor_copy(dn[:], ot[DH:DH + 1, :])
                rec = sp.tile([1, 512], F32, tag="rec")
                nc.vector.reciprocal_approx_fast(rec[:], dn[:])
                rb = sp.tile([DH, 512], F32, tag="rb")
                nc.gpsimd.partition_broadcast(rb[:], rec[:])
                r0 = 64 * (h % 2)
                dst = on_[r0:r0 + 64, h // 2, qsl]
                nc.vector.tensor_tensor(dst, ot[0:DH, :], rb[:], ALU.mult)

            def emit_c_pair(qb, h0, h1):
                gens = [gen_c(qb, h0), gen_c(qb, h1)]
                alive = [True, True]
                while any(alive):
                    for i in (0, 1):
                        if alive[i]:
                            try:
                                next(gens[i])
                            except StopIteration:
                                alive[i] = False

            # ---- output projection for q-block qb: out^T[j, sq] ----
            def emit_proj(qb, jts):
                for jt in jts:
                    psjt = ps_pair.tile([128, 1024], F32, space="PSUM",
                                        tag="pair")
                    psj = psjt[:, 0:512]
                    for co in range(4):
                        nc.tensor.matmul(
                            psj, wp[:, co, jt * 128:(jt + 1) * 128],
                            on_[:, co, qb * 512:(qb + 1) * 512],
                            start=(co == 0), stop=(co == 3))
                    ot_sb = op.tile([128, 512], F32, tag="o")
                    nc.vector.tensor_scalar_add(ot_sb[:], psj,
                                                bps[:, jt:jt + 1])
                    nc.sync.dma_start(
                        out_d[jt * 128:(jt + 1) * 128,
                              qb * 512:(qb + 1) * 512],
                        ot_sb[:])

            # ---- emission: software-pipeline B(qb+1) and proj(qb-1) into
            # the attention loop over (qb, head-pair) ----
            emit_xdma(0)
            # sb=0: emit only what head-pair (0,1) needs (Q0, K0, all V),
            # then start attention while the remaining Q/K groups stream in
            for gi in (0, 4, 8, 9, 10, 11):
                emit_b_group(0, gi)
            b0_rest = [(1, 5), (2, 6), (3, 7), ()]
            for qb in range(4):
                if qb < 3:
                    emit_xdma(qb + 1)
                for hp in range(4):  # head pairs (2hp, 2hp+1)
                    if qb == 0 and hp < 3:
                        for gi in b0_rest[hp]:
                            emit_b_group(0, gi)
                    if qb < 3:
                        for gi in range(3 * hp, 3 * hp + 3):
                            emit_b_group(qb + 1, gi)
                    emit_c_pair(qb, 2 * hp, 2 * hp + 1)
                    if qb >= 1:
                        emit_proj(qb - 1, [2 * hp, 2 * hp + 1])
            emit_proj(3, range(8))

    nc.compile()
    return nc


def kernel(x, Wq, bq, Wk, bk, Wv, bv, Wp, bp):
    global _NC
    if _NC is None:
        _NC = _build()

    x = np.asarray(x, np.float32)
    Wq, bq = np.asarray(Wq, np.float32), np.asarray(bq, np.float32)
    Wk, bk = np.asarray(Wk, np.float32), np.asarray(bk, np.float32)
    Wv, bv = np.asarray(Wv, np.float32), np.asarray(bv, np.float32)
    Wp, bp = np.asarray(Wp, np.float32), np.asarray(bp, np.float32)

    bf = ml_dtypes.bfloat16
    negI = np.ascontiguousarray((-30.0 * np.eye(128, dtype=np.float32)).astype(bf))
    i_ = np.arange(128)
    ltB = np.ascontiguousarray(
        (i_[None, :] < i_[:, None]).astype(np.float32).astype(bf))

    xbT = [np.ascontiguousarray(x[b].T.astype(bf)) for b in range(B)]

    in_maps = []
    for c in range(N_CORES):
        b, g = c // 2, c % 2
        hs = range(8 * g, 8 * g + 8)
        kidx = np.array([dh * 16 + h for h in hs for dh in range(DH)])
        fsl = slice(FG * g, FG * (g + 1))
        bp_c = (bp if g == 0 else 0.0) + Wp[:, fsl] @ bv[fsl]
        in_maps.append({
            "xbT": xbT[b],
            "wqT": np.ascontiguousarray((SCALE * Wq[fsl].T).astype(bf)),
            "wkT": np.ascontiguousarray(Wk[kidx].T.astype(bf)),
            "wvT": np.ascontiguousarray(Wv[fsl].T.astype(bf)),
            "wpT": np.ascontiguousarray(Wp[:, fsl].T.astype(bf)),
            "bqs": np.ascontiguousarray((SCALE * bq[fsl]).reshape(4, 128).T),
            "bks": np.ascontiguousarray(bk[kidx].reshape(4, 128).T),
            "bps": np.ascontiguousarray(bp_c.reshape(8, 128).T.astype(np.float32)),
            "negI": negI,
            "ltB": ltB,
        })

    res = run_bass_kernel_spmd(_NC, in_maps, core_ids=list(range(N_CORES)))
    out = np.empty((B, S, D), np.float32)
    for b in range(B):
        acc = res.results[2 * b]["outT"] + res.results[2 * b + 1]["outT"]
        out[b] = acc.T
    return out


# revision 32
# speedup vs baseline: 1.0118x; 1.0118x over previous
"""Multi-head self-attention (B=4,S=2048,D=1024,H=16,DH=64, causal) on 8 trn2 cores.

Sharding: core c -> batch b=c//2, head-group g=c%2 (8 heads each).
Per-core pipeline:
- QKV projections in bf16 (PE), streamed per 512-wide s block and software-
  pipelined against attention of the previous q-block.
- Scores in bf16. (fp8 DoubleRow was tried and measured SLOWER: fp8 matmuls
  pay ~150-210ns PE transition penalties that eat the 2x column-rate gain.)
- Causal mask added in PSUM by a small [-30*I] @ [c<p] matmul on the 128-wide
  diagonal strip; above-diagonal columns of diagonal tiles are skipped.
- exp on ACT engine batched over k-tile pairs ([128,1024] two-bank PSUM reads).
- attn@V in bf16 with ones-column denominator; renorm via
  reciprocal_approx_fast (DVE) + partition_broadcast (gpsimd) + mult (DVE).
- bv and bp folded host-side into the output-projection bias.
- Two heads' attention streams interleaved to widen the exp-shadow window.
Host sums the two head-group partial outputs per batch.

K-projection quirk (reference views k as (B,S,DH,H)): head h uses Wk rows
[dh*16+h for dh in range(64)] -- handled by host-side row gather.
"""
import numpy as np
import ml_dtypes

import concourse.mybir as mybir
import concourse.tile as tile
from concourse import bacc
from concourse.bass_utils import run_bass_kernel_spmd

F32 = mybir.dt.float32
BF16 = mybir.dt.bfloat16
AF = mybir.ActivationFunctionType
ALU = mybir.AluOpType

B, S, D, H, DH = 4, 2048, 1024, 16, 64
FG = 512          # features per head-group (8 heads * 64)
N_CORES = 8
SCALE = 0.125     # 1/sqrt(64)

_NC = None


def _build():
    nc = bacc.Bacc("TRN2", target_bir_lowering=False, debug=False,
                   num_devices=N_CORES, enable_asserts=False)
    xbT_d = nc.dram_tensor("xbT", [D, S], BF16, kind="ExternalInput").ap()
    wqT_d = nc.dram_tensor("wqT", [D, FG], BF16, kind="ExternalInput").ap()
    wkT_d = nc.dram_tensor("wkT", [D, FG], BF16, kind="ExternalInput").ap()
    wvT_d = nc.dram_tensor("wvT", [D, FG], BF16, kind="ExternalInput").ap()
    wpT_d = nc.dram_tensor("wpT", [FG, D], BF16, kind="ExternalInput").ap()
    bqs_d = nc.dram_tensor("bqs", [128, 4], F32, kind="ExternalInput").ap()
    bks_d = nc.dram_tensor("bks", [128, 4], F32, kind="ExternalInput").ap()
    bps_d = nc.dram_tensor("bps", [128, 8], F32, kind="ExternalInput").ap()
    negI_d = nc.dram_tensor("negI", [128, 128], BF16, kind="ExternalInput").ap()
    ltB_d = nc.dram_tensor("ltB", [128, 128], BF16, kind="ExternalInput").ap()
    out_d = nc.dram_tensor("outT", [D, S], F32, kind="ExternalOutput").ap()

    with tile.TileContext(nc) as tc:
        with tc.tile_pool(name="persist", bufs=1) as pp, \
             tc.tile_pool(name="xin", bufs=2) as xp, \
             tc.tile_pool(name="etile", bufs=4) as ep, \
             tc.tile_pool(name="small", bufs=4) as sp, \
             tc.tile_pool(name="outtile", bufs=3) as op, \
             tc.tile_pool(name="pspair", bufs=3, space="PSUM") as ps_pair, \
             tc.tile_pool(name="psot", bufs=2, space="PSUM") as ps_ot:

            # ---- persistent SBUF tensors ----
            wq = pp.tile([128, 8, FG], BF16)   # [dp, do, f]  (pre-scaled 1/8)
            wk = pp.tile([128, 8, FG], BF16)
            wv = pp.tile([128, 8, FG], BF16)
            wp = pp.tile([128, 4, D], BF16)    # [cp, co, j]
            qt = pp.tile([128, 4, S], BF16)    # [fp, fo, s]
            kt = pp.tile([128, 4, S], BF16)
            va = pp.tile([128, 16, 8, DH + 1], BF16)  # [skp, sko, h, dh|1]
            on_ = pp.tile([128, 4, S], BF16)   # renormed out^T  [cp, co, s]
            negI = pp.tile([128, 128], BF16)
            ltB = pp.tile([128, 128], BF16)
            bqs = pp.tile([128, 4], F32)
            bks = pp.tile([128, 4], F32)
            bps = pp.tile([128, 8], F32)

            nc.gpsimd.dma_start(wq[:], wqT_d.rearrange("(do dp) f -> dp do f", dp=128))
            nc.gpsimd.dma_start(wk[:], wkT_d.rearrange("(do dp) f -> dp do f", dp=128))
            nc.gpsimd.dma_start(wv[:], wvT_d.rearrange("(do dp) f -> dp do f", dp=128))
            nc.gpsimd.dma_start(wp[:], wpT_d.rearrange("(co cp) j -> cp co j", cp=128))
            nc.sync.dma_start(negI[:], negI_d[:])
            nc.sync.dma_start(ltB[:], ltB_d[:])
            nc.sync.dma_start(bqs[:], bqs_d[:])
            nc.sync.dma_start(bks[:], bks_d[:])
            nc.sync.dma_start(bps[:], bps_d[:])
            nc.vector.memset(va[:, :, :, DH:DH + 1], 1.0)

            xbT_r = xbT_d.rearrange("(do dp) s -> dp do s", dp=128)

            xtiles = {}

            def emit_xdma(sb):
                xb = xp.tile([128, 8, 512], BF16)
                nc.gpsimd.dma_start(xb[:], xbT_r[:, :, sb * 512:(sb + 1) * 512])
                xtiles[sb] = xb

            # ---- phase B: QKV projections for one 512-wide s block ----
            # group index: 0-3 Q(ft), 4-7 K(ft), 8-11 V(st)
            def emit_b_group(sb, gi):
                xb = xtiles[sb]
                ssl = slice(sb * 512, (sb + 1) * 512)
                if gi < 8:  # Q or K projection, output [f=128, s=512]
                    ft = gi % 4
                    w_sb = wq if gi < 4 else wk
                    pst = ps_pair.tile([128, 1024], F32, space="PSUM",
                                       tag="pair")
                    ps = pst[:, 0:512]
                    for do in range(8):
                        nc.tensor.matmul(
                            ps, w_sb[:, do, ft * 128:(ft + 1) * 128],
                            xb[:, do, :], start=(do == 0), stop=(do == 7))
                    dst, bias = (qt, bqs) if gi < 4 else (kt, bks)
                    nc.vector.tensor_scalar_add(
                        dst[:, ft, ssl], ps, bias[:, ft:ft + 1])
                else:       # V projection, output [s=128, f=512]
                    st = gi - 8
                    pst = ps_pair.tile([128, 1024], F32, space="PSUM",
                                       tag="pair")
                    ps = pst[:, 0:512]
                    for do in range(8):
                        nc.tensor.matmul(
                            ps, xtiles[sb][:, do, st * 128:(st + 1) * 128],
                            wv[:, do, :], start=(do == 0), stop=(do == 7))
                    nc.vector.tensor_copy(
                        va[:, sb * 4 + st, :, :DH],
                        ps.rearrange("p (h d) -> p h d", h=8))

            # ---- bf16 score matmul for head h, k-tile t, q-block qb ----
            def mm_score(out_ap, h, t, qb, c0, start, stop):
                g2, j = h % 2, h // 2
                p0 = 64 * g2
                lhsT = kt[p0:p0 + 64, j, 128 * t:128 * t + 128]
                rhs = qt[p0:p0 + 64, j, qb * 512 + c0:(qb + 1) * 512]
                nc.tensor.matmul(out_ap, lhsT, rhs, start=start, stop=stop)

            # ---- attention for (q-block qb, head h), generator-staged so
            # two heads' streams can be interleaved (widens the window for
            # hiding exp latency behind the other head's matmuls) ----
            def emit_av(ot, qb, h, et, u, nt):
                for half in range(2):
                    t = 2 * u + half
                    m = t - 4 * qb
                    c0 = 0 if m < 0 else 128 * m
                    hsl = 512 * half
                    nc.tensor.matmul(
                        ot[0:DH + 1, c0:512], va[:, t, h, :],
                        et[:, hsl + c0:hsl + 512],
                        start=(t == 0), stop=(t == nt - 1),
                        skip_group_check=True)

            def gen_c(qb, h):
                nt = 4 * qb + 4
                qsl = slice(qb * 512, (qb + 1) * 512)
                ot = ps_ot.tile([DH + 1, 512], F32, space="PSUM", tag="ot")
                prev = None  # (et, u) whose attn@V is deferred one round
                for u in range(nt // 2):
                    pt = ps_pair.tile([128, 1024], F32, space="PSUM", tag="pair")
                    et = ep.tile([128, 1024], BF16, tag="e")
                    for half in range(2):
                        t = 2 * u + half
                        m = t - 4 * qb
                        hsl = 512 * half
                        if m < 0:  # full tile
                            mm_score(pt[:, hsl:hsl + 512], h, t, qb, 0,
                                     True, True)
                        else:      # diagonal tile: trim cols, add mask strip
                            c0 = 128 * m
                            mm_score(pt[:, hsl + c0:hsl + 512], h, t, qb, c0,
                                     True, False)
                            nc.tensor.matmul(
                                pt[:, hsl + c0:hsl + c0 + 128], negI[:],
                                ltB[:], start=False, stop=True,
                                skip_group_check=True)
                    yield
                    # exp (ACT), batched over the pair when both halves full
                    m0 = 2 * u - 4 * qb
                    if m0 < 0:
                        nc.scalar.activation(et[:], pt[:], AF.Exp)
                    else:
                        c0a, c0b = 128 * m0, 128 * (m0 + 1)
                        nc.scalar.activation(
                            et[:, c0a:512], pt[:, c0a:512], AF.Exp)
                        nc.scalar.activation(
                            et[:, 512 + c0b:1024], pt[:, 512 + c0b:1024],
                            AF.Exp)
                    if prev is not None:
                        emit_av(ot, qb, h, prev[0], prev[1], nt)
                    prev = (et, u)
                    yield
                emit_av(ot, qb, h, prev[0], prev[1], nt)
                # softmax renorm: divide by ones-column row of ot
                dn = sp.tile([1, 512], F32, tag="dn")
                nc.vector.tensor_copy(dn[:], ot[DH:DH + 1, :])
                rec = sp.tile([1, 512], F32, tag="rec")
                nc.vector.reciprocal_approx_fast(rec[:], dn[:])
                rb = sp.tile([DH, 512], F32, tag="rb")
                nc.gpsimd.partition_broadcast(rb[:], rec[:])
                r0 = 64 * (h % 2)
                dst = on_[r0:r0 + 64, h // 2, qsl]
                nc.vector.tensor_tensor(dst, ot[0:DH, :], rb[:], ALU.mult)

            def emit_c_pair(qb, h0, h1):
                gens = [gen_c(qb, h0), gen_c(qb, h1)]
                alive = [True, True]
                while any(alive):
                    for i in (0, 1):
                        if alive[i]:
                            try:
                                next(gens[i])
                            except StopIteration:
                                alive[i] = False

            # ---- output projection for q-block qb: out^T[j, sq] ----
            def emit_proj(qb, jts):
                for jt in jts:
                    psjt = ps_pair.tile([128, 1024], F32, space="PSUM",
                                        tag="pair")
                    psj = psjt[:, 0:512]
                    for co in range(4):
                        nc.tensor.matmul(
                            psj, wp[:, co, jt * 128:(jt + 1) * 128],
                            on_[:, co, qb * 512:(qb + 1) * 512],
                            start=(co == 0), stop=(co == 3))
                    ot_sb = op.tile([128, 512], F32, tag="o")
                    nc.vector.tensor_scalar_add(ot_sb[:], psj,
                                                bps[:, jt:jt + 1])
                    nc.sync.dma_start(
                        out_d[jt * 128:(jt + 1) * 128,
                              qb * 512:(qb + 1) * 512],
                        ot_sb[:])

            # ---- emission: software-pipeline B(qb+1) and proj(qb-1) into
            # the attention loop over (qb, head-pair) ----
            emit_xdma(0)
            for gi in range(12):
                emit_b_group(0, gi)
            for qb in range(4):
                if qb < 3:
                    emit_xdma(qb + 1)
                for hp in range(4):  # head pairs (2hp, 2hp+1)
                    if qb < 3:
                        for gi in range(3 * hp, 3 * hp + 3):
                            emit_b_group(qb + 1, gi)
                    emit_c_pair(qb, 2 * hp, 2 * hp + 1)
                    if qb >= 1:
                        emit_proj(qb - 1, [2 * hp, 2 * hp + 1])
            emit_proj(3, range(8))

    nc.compile()
    return nc


def kernel(x, Wq, bq, Wk, bk, Wv, bv, Wp, bp):
    global _NC
    if _NC is None:
        _NC = _build()

    x = np.asarray(x, np.float32)
    Wq, bq = np.asarray(Wq, np.float32), np.asarray(bq, np.float32)
    Wk, bk = np.asarray(Wk, np.float32), np.asarray(bk, np.float32)
    Wv, bv = np.asarray(Wv, np.float32), np.asarray(bv, np.float32)
    Wp, bp = np.asarray(Wp, np.float32), np.asarray(bp, np.float32)

    bf = ml_dtypes.bfloat16
    negI = np.ascontiguousarray((-30.0 * np.eye(128, dtype=np.float32)).astype(bf))
    i_ = np.arange(128)
    ltB = np.ascontiguousarray(
        (i_[None, :] < i_[:, None]).astype(np.float32).astype(bf))

    xbT = [np.ascontiguousarray(x[b].T.astype(bf)) for b in range(B)]

    in_maps = []
    for c in range(N_CORES):
        b, g = c // 2, c % 2
        hs = range(8 * g, 8 * g + 8)
        kidx = np.array([dh * 16 + h for h in hs for dh in range(DH)])
        fsl = slice(FG * g, FG * (g + 1))
        bp_c = (bp if g == 0 else 0.0) + Wp[:, fsl] @ bv[fsl]
        in_maps.append({
            "xbT": xbT[b],
            "wqT": np.ascontiguousarray((SCALE * Wq[fsl].T).astype(bf)),
            "wkT": np.ascontiguousarray(Wk[kidx].T.astype(bf)),
            "wvT": np.ascontiguousarray(Wv[fsl].T.astype(bf)),
            "wpT": np.ascontiguousarray(Wp[:, fsl].T.astype(bf)),
            "bqs": np.ascontiguousarray((SCALE * bq[fsl]).reshape(4, 128).T),
            "bks": np.ascontiguousarray(bk[kidx].reshape(4, 128).T),
            "bps": np.ascontiguousarray(bp_c.reshape(8, 128).T.astype(np.float32)),
            "negI": negI,
            "ltB": ltB,
        })

    res = run_bass_kernel_spmd(_NC, in_maps, core_ids=list(range(N_CORES)))
    out = np.empty((B, S, D), np.float32)
    for b in range(B):
        acc = res.results[2 * b]["outT"] + res.results[2 * b + 1]["outT"]
        out[b] = acc.T
    return out


# revision 33
# speedup vs baseline: 1.0122x; 1.0004x over previous
"""Multi-head self-attention (B=4,S=2048,D=1024,H=16,DH=64, causal) on 8 trn2 cores.

Sharding: core c -> batch b=c//2, head-group g=c%2 (8 heads each).
Per-core pipeline:
- QKV projections in bf16 (PE), streamed per 512-wide s block and software-
  pipelined against attention of the previous q-block.
- Scores in bf16. (fp8 DoubleRow was tried and measured SLOWER: fp8 matmuls
  pay ~150-210ns PE transition penalties that eat the 2x column-rate gain.)
- Causal mask added in PSUM by a small [-30*I] @ [c<p] matmul on the 128-wide
  diagonal strip; above-diagonal columns of diagonal tiles are skipped.
- exp on ACT engine batched over k-tile pairs ([128,1024] two-bank PSUM reads).
- attn@V in bf16 with ones-column denominator; renorm via
  reciprocal_approx_fast (DVE) + partition_broadcast (gpsimd) + mult (DVE).
- bv and bp folded host-side into the output-projection bias.
- Two heads' attention streams interleaved to widen the exp-shadow window.
Host sums the two head-group partial outputs per batch.

K-projection quirk (reference views k as (B,S,DH,H)): head h uses Wk rows
[dh*16+h for dh in range(64)] -- handled by host-side row gather.
"""
import numpy as np
import ml_dtypes

import concourse.mybir as mybir
import concourse.tile as tile
from concourse import bacc
from concourse.bass_utils import run_bass_kernel_spmd

F32 = mybir.dt.float32
BF16 = mybir.dt.bfloat16
AF = mybir.ActivationFunctionType
ALU = mybir.AluOpType

B, S, D, H, DH = 4, 2048, 1024, 16, 64
FG = 512          # features per head-group (8 heads * 64)
N_CORES = 8
SCALE = 0.125     # 1/sqrt(64)

_NC = None


def _build():
    nc = bacc.Bacc("TRN2", target_bir_lowering=False, debug=False,
                   num_devices=N_CORES, enable_asserts=False)
    xbT_d = nc.dram_tensor("xbT", [D, S], BF16, kind="ExternalInput").ap()
    wqT_d = nc.dram_tensor("wqT", [D, FG], BF16, kind="ExternalInput").ap()
    wkT_d = nc.dram_tensor("wkT", [D, FG], BF16, kind="ExternalInput").ap()
    wvT_d = nc.dram_tensor("wvT", [D, FG], BF16, kind="ExternalInput").ap()
    wpT_d = nc.dram_tensor("wpT", [FG, D], BF16, kind="ExternalInput").ap()
    bqs_d = nc.dram_tensor("bqs", [128, 4], F32, kind="ExternalInput").ap()
    bks_d = nc.dram_tensor("bks", [128, 4], F32, kind="ExternalInput").ap()
    bps_d = nc.dram_tensor("bps", [128, 8], F32, kind="ExternalInput").ap()
    negI_d = nc.dram_tensor("negI", [128, 128], BF16, kind="ExternalInput").ap()
    ltB_d = nc.dram_tensor("ltB", [128, 128], BF16, kind="ExternalInput").ap()
    out_d = nc.dram_tensor("outT", [D, S], F32, kind="ExternalOutput").ap()

    with tile.TileContext(nc) as tc:
        with tc.tile_pool(name="persist", bufs=1) as pp, \
             tc.tile_pool(name="xin", bufs=2) as xp, \
             tc.tile_pool(name="etile", bufs=6) as ep, \
             tc.tile_pool(name="small", bufs=4) as sp, \
             tc.tile_pool(name="outtile", bufs=4) as op, \
             tc.tile_pool(name="pspair", bufs=3, space="PSUM") as ps_pair, \
             tc.tile_pool(name="psot", bufs=2, space="PSUM") as ps_ot:

            # ---- persistent SBUF tensors ----
            wq = pp.tile([128, 8, FG], BF16)   # [dp, do, f]  (pre-scaled 1/8)
            wk = pp.tile([128, 8, FG], BF16)
            wv = pp.tile([128, 8, FG], BF16)
            wp = pp.tile([128, 4, D], BF16)    # [cp, co, j]
            qt = pp.tile([128, 4, S], BF16)    # [fp, fo, s]
            kt = pp.tile([128, 4, S], BF16)
            va = pp.tile([128, 16, 8, DH + 1], BF16)  # [skp, sko, h, dh|1]
            on_ = pp.tile([128, 4, S], BF16)   # renormed out^T  [cp, co, s]
            negI = pp.tile([128, 128], BF16)
            ltB = pp.tile([128, 128], BF16)
            bqs = pp.tile([128, 4], F32)
            bks = pp.tile([128, 4], F32)
            bps = pp.tile([128, 8], F32)

            nc.gpsimd.dma_start(wq[:], wqT_d.rearrange("(do dp) f -> dp do f", dp=128))
            nc.gpsimd.dma_start(wk[:], wkT_d.rearrange("(do dp) f -> dp do f", dp=128))
            nc.gpsimd.dma_start(wv[:], wvT_d.rearrange("(do dp) f -> dp do f", dp=128))
            nc.gpsimd.dma_start(wp[:], wpT_d.rearrange("(co cp) j -> cp co j", cp=128))
            nc.sync.dma_start(negI[:], negI_d[:])
            nc.sync.dma_start(ltB[:], ltB_d[:])
            nc.sync.dma_start(bqs[:], bqs_d[:])
            nc.sync.dma_start(bks[:], bks_d[:])
            nc.sync.dma_start(bps[:], bps_d[:])
            nc.vector.memset(va[:, :, :, DH:DH + 1], 1.0)

            xbT_r = xbT_d.rearrange("(do dp) s -> dp do s", dp=128)

            xtiles = {}

            def emit_xdma(sb):
                xb = xp.tile([128, 8, 512], BF16)
                nc.gpsimd.dma_start(xb[:], xbT_r[:, :, sb * 512:(sb + 1) * 512])
                xtiles[sb] = xb

            # ---- phase B: QKV projections for one 512-wide s block ----
            # group index: 0-3 Q(ft), 4-7 K(ft), 8-11 V(st)
            def emit_b_group(sb, gi):
                xb = xtiles[sb]
                ssl = slice(sb * 512, (sb + 1) * 512)
                if gi < 8:  # Q or K projection, output [f=128, s=512]
                    ft = gi % 4
                    w_sb = wq if gi < 4 else wk
                    pst = ps_pair.tile([128, 1024], F32, space="PSUM",
                                       tag="pair")
                    ps = pst[:, 0:512]
                    for do in range(8):
                        nc.tensor.matmul(
                            ps, w_sb[:, do, ft * 128:(ft + 1) * 128],
                            xb[:, do, :], start=(do == 0), stop=(do == 7))
                    dst, bias = (qt, bqs) if gi < 4 else (kt, bks)
                    nc.vector.tensor_scalar_add(
                        dst[:, ft, ssl], ps, bias[:, ft:ft + 1])
                else:       # V projection, output [s=128, f=512]
                    st = gi - 8
                    pst = ps_pair.tile([128, 1024], F32, space="PSUM",
                                       tag="pair")
                    ps = pst[:, 0:512]
                    for do in range(8):
                        nc.tensor.matmul(
                            ps, xtiles[sb][:, do, st * 128:(st + 1) * 128],
                            wv[:, do, :], start=(do == 0), stop=(do == 7))
                    nc.vector.tensor_copy(
                        va[:, sb * 4 + st, :, :DH],
                        ps.rearrange("p (h d) -> p h d", h=8))

            # ---- bf16 score matmul for head h, k-tile t, q-block qb ----
            def mm_score(out_ap, h, t, qb, c0, start, stop):
                g2, j = h % 2, h // 2
                p0 = 64 * g2
                lhsT = kt[p0:p0 + 64, j, 128 * t:128 * t + 128]
                rhs = qt[p0:p0 + 64, j, qb * 512 + c0:(qb + 1) * 512]
                nc.tensor.matmul(out_ap, lhsT, rhs, start=start, stop=stop)

            # ---- attention for (q-block qb, head h), generator-staged so
            # two heads' streams can be interleaved (widens the window for
            # hiding exp latency behind the other head's matmuls) ----
            def emit_av(ot, qb, h, et, u, nt):
                for half in range(2):
                    t = 2 * u + half
                    m = t - 4 * qb
                    c0 = 0 if m < 0 else 128 * m
                    hsl = 512 * half
                    nc.tensor.matmul(
                        ot[0:DH + 1, c0:512], va[:, t, h, :],
                        et[:, hsl + c0:hsl + 512],
                        start=(t == 0), stop=(t == nt - 1),
                        skip_group_check=True)

            def gen_c(qb, h):
                nt = 4 * qb + 4
                qsl = slice(qb * 512, (qb + 1) * 512)
                ot = ps_ot.tile([DH + 1, 512], F32, space="PSUM", tag="ot")
                prev = None  # (et, u) whose attn@V is deferred one round
                for u in range(nt // 2):
                    pt = ps_pair.tile([128, 1024], F32, space="PSUM", tag="pair")
                    et = ep.tile([128, 1024], BF16, tag="e")
                    for half in range(2):
                        t = 2 * u + half
                        m = t - 4 * qb
                        hsl = 512 * half
                        if m < 0:  # full tile
                            mm_score(pt[:, hsl:hsl + 512], h, t, qb, 0,
                                     True, True)
                        else:      # diagonal tile: trim cols, add mask strip
                            c0 = 128 * m
                            mm_score(pt[:, hsl + c0:hsl + 512], h, t, qb, c0,
                                     True, False)
                            nc.tensor.matmul(
                                pt[:, hsl + c0:hsl + c0 + 128], negI[:],
                                ltB[:], start=False, stop=True,
                                skip_group_check=True)
                    yield
                    # exp (ACT), batched over the pair when both halves full
                    m0 = 2 * u - 4 * qb
                    if m0 < 0:
                        nc.scalar.activation(et[:], pt[:], AF.Exp)
                    else:
                        c0a, c0b = 128 * m0, 128 * (m0 + 1)
                        nc.scalar.activation(
                            et[:, c0a:512], pt[:, c0a:512], AF.Exp)
                        nc.scalar.activation(
                            et[:, 512 + c0b:1024], pt[:, 512 + c0b:1024],
                            AF.Exp)
                    if prev is not None:
                        emit_av(ot, qb, h, prev[0], prev[1], nt)
                    prev = (et, u)
                    yield
                emit_av(ot, qb, h, prev[0], prev[1], nt)
                # softmax renorm: divide by ones-column row of ot
                dn = sp.tile([1, 512], F32, tag="dn")
                nc.vector.tensor_copy(dn[:], ot[DH:DH + 1, :])
                rec = sp.tile([1, 512], F32, tag="rec")
                nc.vector.reciprocal_approx_fast(rec[:], dn[:])
                rb = sp.tile([DH, 512], F32, tag="rb")
                nc.gpsimd.partition_broadcast(rb[:], rec[:])
                r0 = 64 * (h % 2)
                dst = on_[r0:r0 + 64, h // 2, qsl]
                nc.vector.tensor_tensor(dst, ot[0:DH, :], rb[:], ALU.mult)

            def emit_c_pair(qb, h0, h1):
                gens = [gen_c(qb, h0), gen_c(qb, h1)]
                alive = [True, True]
                while any(alive):
                    for i in (0, 1):
                        if alive[i]:
                            try:
                                next(gens[i])
                            except StopIteration:
                                alive[i] = False

            # ---- output projection for q-block qb: out^T[j, sq] ----
            def emit_proj(qb, jts):
                for jt in jts:
                    psjt = ps_pair.tile([128, 1024], F32, space="PSUM",
                                        tag="pair")
                    psj = psjt[:, 0:512]
                    for co in range(4):
                        nc.tensor.matmul(
                            psj, wp[:, co, jt * 128:(jt + 1) * 128],
                            on_[:, co, qb * 512:(qb + 1) * 512],
                            start=(co == 0), stop=(co == 3))
                    ot_sb = op.tile([128, 512], F32, tag="o")
                    nc.vector.tensor_scalar_add(ot_sb[:], psj,
                                                bps[:, jt:jt + 1])
                    nc.sync.dma_start(
                        out_d[jt * 128:(jt + 1) * 128,
                              qb * 512:(qb + 1) * 512],
                        ot_sb[:])

            # ---- emission: software-pipeline B(qb+1) and proj(qb-1) into
            # the attention loop over (qb, head-pair) ----
            emit_xdma(0)
            for gi in range(12):
                emit_b_group(0, gi)
            for qb in range(4):
                if qb < 3:
                    emit_xdma(qb + 1)
                for hp in range(4):  # head pairs (2hp, 2hp+1)
                    if qb < 3:
                        for gi in range(3 * hp, 3 * hp + 3):
                            emit_b_group(qb + 1, gi)
                    emit_c_pair(qb, 2 * hp, 2 * hp + 1)
                    if qb >= 1:
                        emit_proj(qb - 1, [2 * hp, 2 * hp + 1])
            emit_proj(3, range(8))

    nc.compile()
    return nc


def kernel(x, Wq, bq, Wk, bk, Wv, bv, Wp, bp):
    global _NC
    if _NC is None:
        _NC = _build()

    x = np.asarray(x, np.float32)
    Wq, bq = np.asarray(Wq, np.float32), np.asarray(bq, np.float32)
    Wk, bk = np.asarray(Wk, np.float32), np.asarray(bk, np.float32)
    Wv, bv = np.asarray(Wv, np.float32), np.asarray(bv, np.float32)
    Wp, bp = np.asarray(Wp, np.float32), np.asarray(bp, np.float32)

    bf = ml_dtypes.bfloat16
    negI = np.ascontiguousarray((-30.0 * np.eye(128, dtype=np.float32)).astype(bf))
    i_ = np.arange(128)
    ltB = np.ascontiguousarray(
        (i_[None, :] < i_[:, None]).astype(np.float32).astype(bf))

    xbT = [np.ascontiguousarray(x[b].T.astype(bf)) for b in range(B)]

    in_maps = []
    for c in range(N_CORES):
        b, g = c // 2, c % 2
        hs = range(8 * g, 8 * g + 8)
        kidx = np.array([dh * 16 + h for h in hs for dh in range(DH)])
        fsl = slice(FG * g, FG * (g + 1))
        bp_c = (bp if g == 0 else 0.0) + Wp[:, fsl] @ bv[fsl]
        in_maps.append({
            "xbT": xbT[b],
            "wqT": np.ascontiguousarray((SCALE * Wq[fsl].T).astype(bf)),
            "wkT": np.ascontiguousarray(Wk[kidx].T.astype(bf)),
            "wvT": np.ascontiguousarray(Wv[fsl].T.astype(bf)),
            "wpT": np.ascontiguousarray(Wp[:, fsl].T.astype(bf)),
            "bqs": np.ascontiguousarray((SCALE * bq[fsl]).reshape(4, 128).T),
            "bks": np.ascontiguousarray(bk[kidx].reshape(4, 128).T),
            "bps": np.ascontiguousarray(bp_c.reshape(8, 128).T.astype(np.float32)),
            "negI": negI,
            "ltB": ltB,
        })

    res = run_bass_kernel_spmd(_NC, in_maps, core_ids=list(range(N_CORES)))
    out = np.empty((B, S, D), np.float32)
    for b in range(B):
        acc = res.results[2 * b]["outT"] + res.results[2 * b + 1]["outT"]
        out[b] = acc.T
    return out


# revision 34
# speedup vs baseline: 1.0434x; 1.0309x over previous
"""Multi-head self-attention (B=4,S=2048,D=1024,H=16,DH=64, causal) on 8 trn2 cores.

Sharding: core c -> batch b=c//2, head-group g=c%2 (8 heads each).
Per-core pipeline:
- QKV projections in bf16 (PE), streamed per 512-wide s block and software-
  pipelined against attention of the previous q-block.
- Scores in bf16. (fp8 DoubleRow was tried and measured SLOWER: fp8 matmuls
  pay ~150-210ns PE transition penalties that eat the 2x column-rate gain.)
- Causal mask added in PSUM by a small [-30*I] @ [c<p] matmul on the 128-wide
  diagonal strip; above-diagonal columns of diagonal tiles are skipped.
- exp on ACT engine batched over k-tile pairs ([128,1024] two-bank PSUM reads).
- attn@V in bf16 with ones-column denominator; renorm via
  reciprocal_approx_fast (DVE) + partition_broadcast (gpsimd) + mult (DVE).
- bv and bp folded host-side into the output-projection bias.
- Two heads' attention streams interleaved to widen the exp-shadow window.
Host sums the two head-group partial outputs per batch.

K-projection quirk (reference views k as (B,S,DH,H)): head h uses Wk rows
[dh*16+h for dh in range(64)] -- handled by host-side row gather.
"""
import numpy as np
import ml_dtypes

import concourse.mybir as mybir
import concourse.tile as tile
from concourse import bacc
from concourse.bass_utils import run_bass_kernel_spmd

F32 = mybir.dt.float32
BF16 = mybir.dt.bfloat16
AF = mybir.ActivationFunctionType
ALU = mybir.AluOpType

B, S, D, H, DH = 4, 2048, 1024, 16, 64
FG = 512          # features per head-group (8 heads * 64)
N_CORES = 8
SCALE = 0.125     # 1/sqrt(64)

_NC = None


def _build():
    nc = bacc.Bacc("TRN2", target_bir_lowering=False, debug=False,
                   num_devices=N_CORES, enable_asserts=False)
    xbT_d = nc.dram_tensor("xbT", [D, S], BF16, kind="ExternalInput").ap()
    wqT_d = nc.dram_tensor("wqT", [D, FG], BF16, kind="ExternalInput").ap()
    wkT_d = nc.dram_tensor("wkT", [D, FG], BF16, kind="ExternalInput").ap()
    wvT_d = nc.dram_tensor("wvT", [D, FG], BF16, kind="ExternalInput").ap()
    wpT_d = nc.dram_tensor("wpT", [FG, D], BF16, kind="ExternalInput").ap()
    bqs_d = nc.dram_tensor("bqs", [128, 4], F32, kind="ExternalInput").ap()
    bks_d = nc.dram_tensor("bks", [128, 4], F32, kind="ExternalInput").ap()
    bps_d = nc.dram_tensor("bps", [128, 8], F32, kind="ExternalInput").ap()
    negI_d = nc.dram_tensor("negI", [128, 128], BF16, kind="ExternalInput").ap()
    ltB_d = nc.dram_tensor("ltB", [128, 128], BF16, kind="ExternalInput").ap()
    out_d = nc.dram_tensor("outT", [D, S], F32, kind="ExternalOutput").ap()

    with tile.TileContext(nc) as tc:
        with tc.tile_pool(name="persist", bufs=1) as pp, \
             tc.tile_pool(name="xin", bufs=2) as xp, \
             tc.tile_pool(name="etile", bufs=6) as ep, \
             tc.tile_pool(name="small", bufs=4) as sp, \
             tc.tile_pool(name="outtile", bufs=4) as op, \
             tc.tile_pool(name="pspair", bufs=3, space="PSUM") as ps_pair, \
             tc.tile_pool(name="psot", bufs=2, space="PSUM") as ps_ot:

            # ---- persistent SBUF tensors ----
            wq = pp.tile([128, 8, FG], BF16)   # [dp, do, f]  (pre-scaled 1/8)
            wk = pp.tile([128, 8, FG], BF16)
            wv = pp.tile([128, 8, FG], BF16)
            wp = pp.tile([128, 4, D], BF16)    # [cp, co, j]
            qt = pp.tile([128, 4, S], BF16)    # [fp, fo, s]
            kt = pp.tile([128, 4, S], BF16)
            va = pp.tile([128, 16, 8, DH + 1], BF16)  # [skp, sko, h, dh|1]
            on_ = pp.tile([128, 4, S], BF16)   # renormed out^T  [cp, co, s]
            negI = pp.tile([128, 128], BF16)
            ltB = pp.tile([128, 128], BF16)
            bqs = pp.tile([128, 4], F32)
            bks = pp.tile([128, 4], F32)
            bps = pp.tile([128, 8], F32)

            xbT_r = xbT_d.rearrange("(do dp) s -> dp do s", dp=128)
            xtiles = {}

            def emit_xdma(sb):
                xb = xp.tile([128, 8, 512], BF16)
                nc.gpsimd.dma_start(xb[:], xbT_r[:, :, sb * 512:(sb + 1) * 512])
                xtiles[sb] = xb

            # x block 0 first in the gpsimd DMA queue: the first Q-proj chain
            # needs only xb0+wq, not the whole weight set
            emit_xdma(0)
            nc.gpsimd.dma_start(wq[:], wqT_d.rearrange("(do dp) f -> dp do f", dp=128))
            nc.gpsimd.dma_start(wk[:], wkT_d.rearrange("(do dp) f -> dp do f", dp=128))
            nc.gpsimd.dma_start(wv[:], wvT_d.rearrange("(do dp) f -> dp do f", dp=128))
            nc.gpsimd.dma_start(wp[:], wpT_d.rearrange("(co cp) j -> cp co j", cp=128))
            nc.sync.dma_start(negI[:], negI_d[:])
            nc.sync.dma_start(ltB[:], ltB_d[:])
            nc.sync.dma_start(bqs[:], bqs_d[:])
            nc.sync.dma_start(bks[:], bks_d[:])
            nc.sync.dma_start(bps[:], bps_d[:])
            nc.vector.memset(va[:, :, :, DH:DH + 1], 1.0)

            # ---- phase B: QKV projections for one 512-wide s block ----
            # group index: 0-3 Q(ft), 4-7 K(ft), 8-11 V(st)
            def emit_b_group(sb, gi):
                xb = xtiles[sb]
                ssl = slice(sb * 512, (sb + 1) * 512)
                if gi < 8:  # Q or K projection, output [f=128, s=512]
                    ft = gi % 4
                    w_sb = wq if gi < 4 else wk
                    pst = ps_pair.tile([128, 1024], F32, space="PSUM",
                                       tag="pair")
                    ps = pst[:, 0:512]
                    for do in range(8):
                        nc.tensor.matmul(
                            ps, w_sb[:, do, ft * 128:(ft + 1) * 128],
                            xb[:, do, :], start=(do == 0), stop=(do == 7))
                    dst, bias = (qt, bqs) if gi < 4 else (kt, bks)
                    nc.vector.tensor_scalar_add(
                        dst[:, ft, ssl], ps, bias[:, ft:ft + 1])
                else:       # V projection, output [s=128, f=512]
                    st = gi - 8
                    pst = ps_pair.tile([128, 1024], F32, space="PSUM",
                                       tag="pair")
                    ps = pst[:, 0:512]
                    for do in range(8):
                        nc.tensor.matmul(
                            ps, xtiles[sb][:, do, st * 128:(st + 1) * 128],
                            wv[:, do, :], start=(do == 0), stop=(do == 7))
                    nc.vector.tensor_copy(
                        va[:, sb * 4 + st, :, :DH],
                        ps.rearrange("p (h d) -> p h d", h=8))

            # ---- bf16 score matmul for head h, k-tile t, q-block qb ----
            def mm_score(out_ap, h, t, qb, c0, start, stop):
                g2, j = h % 2, h // 2
                p0 = 64 * g2
                lhsT = kt[p0:p0 + 64, j, 128 * t:128 * t + 128]
                rhs = qt[p0:p0 + 64, j, qb * 512 + c0:(qb + 1) * 512]
                nc.tensor.matmul(out_ap, lhsT, rhs, start=start, stop=stop)

            # ---- attention for (q-block qb, head h), generator-staged so
            # two heads' streams can be interleaved (widens the window for
            # hiding exp latency behind the other head's matmuls) ----
            def emit_av(ot, qb, h, et, u, nt):
                for half in range(2):
                    t = 2 * u + half
                    m = t - 4 * qb
                    c0 = 0 if m < 0 else 128 * m
                    hsl = 512 * half
                    nc.tensor.matmul(
                        ot[0:DH + 1, c0:512], va[:, t, h, :],
                        et[:, hsl + c0:hsl + 512],
                        start=(t == 0), stop=(t == nt - 1),
                        skip_group_check=True)

            def gen_c(qb, h):
                nt = 4 * qb + 4
                qsl = slice(qb * 512, (qb + 1) * 512)
                ot = ps_ot.tile([DH + 1, 512], F32, space="PSUM", tag="ot")
                prev = None  # (et, u) whose attn@V is deferred one round
                for u in range(nt // 2):
                    pt = ps_pair.tile([128, 1024], F32, space="PSUM", tag="pair")
                    et = ep.tile([128, 1024], BF16, tag="e")
                    for half in range(2):
                        t = 2 * u + half
                        m = t - 4 * qb
                        hsl = 512 * half
                        if m < 0:  # full tile
                            mm_score(pt[:, hsl:hsl + 512], h, t, qb, 0,
                                     True, True)
                        else:      # diagonal tile: trim cols, add mask strip
                            c0 = 128 * m
                            mm_score(pt[:, hsl + c0:hsl + 512], h, t, qb, c0,
                                     True, False)
                            nc.tensor.matmul(
                                pt[:, hsl + c0:hsl + c0 + 128], negI[:],
                                ltB[:], start=False, stop=True,
                                skip_group_check=True)
                    yield
                    # exp (ACT), batched over the pair when both halves full
                    m0 = 2 * u - 4 * qb
                    if m0 < 0:
                        nc.scalar.activation(et[:], pt[:], AF.Exp)
                    else:
                        c0a, c0b = 128 * m0, 128 * (m0 + 1)
                        nc.scalar.activation(
                            et[:, c0a:512], pt[:, c0a:512], AF.Exp)
                        nc.scalar.activation(
                            et[:, 512 + c0b:1024], pt[:, 512 + c0b:1024],
                            AF.Exp)
                    if prev is not None:
                        emit_av(ot, qb, h, prev[0], prev[1], nt)
                    prev = (et, u)
                    yield
                emit_av(ot, qb, h, prev[0], prev[1], nt)
                # softmax renorm: divide by ones-column row of ot
                dn = sp.tile([1, 512], F32, tag="dn")
                nc.vector.tensor_copy(dn[:], ot[DH:DH + 1, :])
                rec = sp.tile([1, 512], F32, tag="rec")
                nc.vector.reciprocal_approx_fast(rec[:], dn[:])
                rb = sp.tile([DH, 512], F32, tag="rb")
                nc.gpsimd.partition_broadcast(rb[:], rec[:])
                r0 = 64 * (h % 2)
                dst = on_[r0:r0 + 64, h // 2, qsl]
                nc.vector.tensor_tensor(dst, ot[0:DH, :], rb[:], ALU.mult)

            def emit_c_pair(qb, h0, h1):
                gens = [gen_c(qb, h0), gen_c(qb, h1)]
                alive = [True, True]
                while any(alive):
                    for i in (0, 1):
                        if alive[i]:
                            try:
                                next(gens[i])
                            except StopIteration:
                                alive[i] = False

            # ---- output projection for q-block qb: out^T[j, sq] ----
            def emit_proj(qb, jts):
                for jt in jts:
                    psjt = ps_pair.tile([128, 1024], F32, space="PSUM",
                                        tag="pair")
                    psj = psjt[:, 0:512]
                    for co in range(4):
                        nc.tensor.matmul(
                            psj, wp[:, co, jt * 128:(jt + 1) * 128],
                            on_[:, co, qb * 512:(qb + 1) * 512],
                            start=(co == 0), stop=(co == 3))
                    ot_sb = op.tile([128, 512], F32, tag="o")
                    nc.vector.tensor_scalar_add(ot_sb[:], psj,
                                                bps[:, jt:jt + 1])
                    nc.sync.dma_start(
                        out_d[jt * 128:(jt + 1) * 128,
                              qb * 512:(qb + 1) * 512],
                        ot_sb[:])

            # ---- emission: software-pipeline B(qb+1) and proj(qb-1) into
            # the attention loop over (qb, head-pair) ----
            for gi in range(12):
                emit_b_group(0, gi)
            for qb in range(4):
                if qb < 3:
                    emit_xdma(qb + 1)
                for hp in range(4):  # head pairs (2hp, 2hp+1)
                    if qb < 3:
                        for gi in range(3 * hp, 3 * hp + 3):
                            emit_b_group(qb + 1, gi)
                    emit_c_pair(qb, 2 * hp, 2 * hp + 1)
                    if qb >= 1:
                        emit_proj(qb - 1, [2 * hp, 2 * hp + 1])
            emit_proj(3, range(8))

    nc.compile()
    return nc


def kernel(x, Wq, bq, Wk, bk, Wv, bv, Wp, bp):
    global _NC
    if _NC is None:
        _NC = _build()

    x = np.asarray(x, np.float32)
    Wq, bq = np.asarray(Wq, np.float32), np.asarray(bq, np.float32)
    Wk, bk = np.asarray(Wk, np.float32), np.asarray(bk, np.float32)
    Wv, bv = np.asarray(Wv, np.float32), np.asarray(bv, np.float32)
    Wp, bp = np.asarray(Wp, np.float32), np.asarray(bp, np.float32)

    bf = ml_dtypes.bfloat16
    negI = np.ascontiguousarray((-30.0 * np.eye(128, dtype=np.float32)).astype(bf))
    i_ = np.arange(128)
    ltB = np.ascontiguousarray(
        (i_[None, :] < i_[:, None]).astype(np.float32).astype(bf))

    xbT = [np.ascontiguousarray(x[b].T.astype(bf)) for b in range(B)]

    in_maps = []
    for c in range(N_CORES):
        b, g = c // 2, c % 2
        hs = range(8 * g, 8 * g + 8)
        kidx = np.array([dh * 16 + h for h in hs for dh in range(DH)])
        fsl = slice(FG * g, FG * (g + 1))
        bp_c = (bp if g == 0 else 0.0) + Wp[:, fsl] @ bv[fsl]
        in_maps.append({
            "xbT": xbT[b],
            "wqT": np.ascontiguousarray((SCALE * Wq[fsl].T).astype(bf)),
            "wkT": np.ascontiguousarray(Wk[kidx].T.astype(bf)),
            "wvT": np.ascontiguousarray(Wv[fsl].T.astype(bf)),
            "wpT": np.ascontiguousarray(Wp[:, fsl].T.astype(bf)),
            "bqs": np.ascontiguousarray((SCALE * bq[fsl]).reshape(4, 128).T),
            "bks": np.ascontiguousarray(bk[kidx].reshape(4, 128).T),
            "bps": np.ascontiguousarray(bp_c.reshape(8, 128).T.astype(np.float32)),
            "negI": negI,
            "ltB": ltB,
        })

    res = run_bass_kernel_spmd(_NC, in_maps, core_ids=list(range(N_CORES)))
    out = np.empty((B, S, D), np.float32)
    for b in range(B):
        acc = res.results[2 * b]["outT"] + res.results[2 * b + 1]["outT"]
        out[b] = acc.T
    return out


# revision 35
# speedup vs baseline: 1.0763x; 1.0316x over previous
"""Multi-head self-attention (B=4,S=2048,D=1024,H=16,DH=64, causal) on 8 trn2 cores.

Sharding: core c -> batch b=c//2, head-group g=c%2 (8 heads each).
Per-core pipeline:
- QKV projections in bf16 (PE), streamed per 512-wide s block and software-
  pipelined against attention of the previous q-block.
- Scores in bf16. (fp8 DoubleRow was tried and measured SLOWER: fp8 matmuls
  pay ~150-210ns PE transition penalties that eat the 2x column-rate gain.)
- Causal mask added in PSUM by a small [-30*I] @ [c<p] matmul on the 128-wide
  diagonal strip; above-diagonal columns of diagonal tiles are skipped.
- exp on ACT engine batched over k-tile pairs ([128,1024] two-bank PSUM reads).
- attn@V in bf16 with ones-column denominator; renorm via
  reciprocal_approx_fast (DVE) + partition_broadcast (gpsimd) + mult (DVE).
- bv and bp folded host-side into the output-projection bias.
- Two heads' attention streams interleaved to widen the exp-shadow window.
Host sums the two head-group partial outputs per batch.

K-projection quirk (reference views k as (B,S,DH,H)): head h uses Wk rows
[dh*16+h for dh in range(64)] -- handled by host-side row gather.
"""
import numpy as np
import ml_dtypes

import concourse.mybir as mybir
import concourse.tile as tile
from concourse import bacc
from concourse.bass_utils import run_bass_kernel_spmd

F32 = mybir.dt.float32
BF16 = mybir.dt.bfloat16
AF = mybir.ActivationFunctionType
ALU = mybir.AluOpType

B, S, D, H, DH = 4, 2048, 1024, 16, 64
FG = 512          # features per head-group (8 heads * 64)
N_CORES = 8
SCALE = 0.125     # 1/sqrt(64)

_NC = None


def _build():
    nc = bacc.Bacc("TRN2", target_bir_lowering=False, debug=False,
                   num_devices=N_CORES, enable_asserts=False)
    xbT_d = nc.dram_tensor("xbT", [D, S], BF16, kind="ExternalInput").ap()
    wqT_d = nc.dram_tensor("wqT", [D, FG], BF16, kind="ExternalInput").ap()
    wkT_d = nc.dram_tensor("wkT", [D, FG], BF16, kind="ExternalInput").ap()
    wvT_d = nc.dram_tensor("wvT", [D, FG], BF16, kind="ExternalInput").ap()
    wpT_d = nc.dram_tensor("wpT", [FG, D], BF16, kind="ExternalInput").ap()
    bqs_d = nc.dram_tensor("bqs", [128, 4], F32, kind="ExternalInput").ap()
    bks_d = nc.dram_tensor("bks", [128, 4], F32, kind="ExternalInput").ap()
    bps_d = nc.dram_tensor("bps", [128, 8], F32, kind="ExternalInput").ap()
    negI_d = nc.dram_tensor("negI", [128, 128], BF16, kind="ExternalInput").ap()
    ltB_d = nc.dram_tensor("ltB", [128, 128], BF16, kind="ExternalInput").ap()
    out_d = nc.dram_tensor("outT", [D, S], F32, kind="ExternalOutput").ap()

    with tile.TileContext(nc) as tc:
        with tc.tile_pool(name="persist", bufs=1) as pp, \
             tc.tile_pool(name="xin", bufs=3) as xp, \
             tc.tile_pool(name="etile", bufs=6) as ep, \
             tc.tile_pool(name="small", bufs=4) as sp, \
             tc.tile_pool(name="outtile", bufs=4) as op, \
             tc.tile_pool(name="pspair", bufs=3, space="PSUM") as ps_pair, \
             tc.tile_pool(name="psot", bufs=2, space="PSUM") as ps_ot:

            # ---- persistent SBUF tensors ----
            wq = pp.tile([128, 8, FG], BF16)   # [dp, do, f]  (pre-scaled 1/8)
            wk = pp.tile([128, 8, FG], BF16)
            wv = pp.tile([128, 8, FG], BF16)
            wp = pp.tile([128, 4, D], BF16)    # [cp, co, j]
            qt = pp.tile([128, 4, S], BF16)    # [fp, fo, s]
            kt = pp.tile([128, 4, S], BF16)
            va = pp.tile([128, 16, 8, DH + 1], BF16)  # [skp, sko, h, dh|1]
            on_ = pp.tile([128, 4, S], BF16)   # renormed out^T  [cp, co, s]
            negI = pp.tile([128, 128], BF16)
            ltB = pp.tile([128, 128], BF16)
            bqs = pp.tile([128, 4], F32)
            bks = pp.tile([128, 4], F32)
            bps = pp.tile([128, 8], F32)

            xbT_r = xbT_d.rearrange("(do dp) s -> dp do s", dp=128)
            xtiles = {}

            def emit_xdma(sb):
                xb = xp.tile([128, 8, 512], BF16)
                nc.gpsimd.dma_start(xb[:], xbT_r[:, :, sb * 512:(sb + 1) * 512])
                xtiles[sb] = xb

            # x block 0 first in the gpsimd DMA queue: the first Q-proj chain
            # needs only xb0+wq, not the whole weight set
            emit_xdma(0)
            nc.gpsimd.dma_start(wq[:], wqT_d.rearrange("(do dp) f -> dp do f", dp=128))
            nc.gpsimd.dma_start(wk[:], wkT_d.rearrange("(do dp) f -> dp do f", dp=128))
            nc.gpsimd.dma_start(wv[:], wvT_d.rearrange("(do dp) f -> dp do f", dp=128))
            nc.gpsimd.dma_start(wp[:], wpT_d.rearrange("(co cp) j -> cp co j", cp=128))
            nc.sync.dma_start(negI[:], negI_d[:])
            nc.sync.dma_start(ltB[:], ltB_d[:])
            nc.sync.dma_start(bqs[:], bqs_d[:])
            nc.sync.dma_start(bks[:], bks_d[:])
            nc.sync.dma_start(bps[:], bps_d[:])
            nc.vector.memset(va[:, :, :, DH:DH + 1], 1.0)

            # ---- phase B: QKV projections for one 512-wide s block ----
            # group index: 0-3 Q(ft), 4-7 K(ft), 8-11 V(st)
            def emit_b_group(sb, gi):
                xb = xtiles[sb]
                ssl = slice(sb * 512, (sb + 1) * 512)
                if gi < 8:  # Q or K projection, output [f=128, s=512]
                    ft = gi % 4
                    w_sb = wq if gi < 4 else wk
                    pst = ps_pair.tile([128, 1024], F32, space="PSUM",
                                       tag="pair")
                    ps = pst[:, 0:512]
                    for do in range(8):
                        nc.tensor.matmul(
                            ps, w_sb[:, do, ft * 128:(ft + 1) * 128],
                            xb[:, do, :], start=(do == 0), stop=(do == 7))
                    dst, bias = (qt, bqs) if gi < 4 else (kt, bks)
                    nc.vector.tensor_scalar_add(
                        dst[:, ft, ssl], ps, bias[:, ft:ft + 1])
                else:       # V projection, output [s=128, f=512]
                    st = gi - 8
                    pst = ps_pair.tile([128, 1024], F32, space="PSUM",
                                       tag="pair")
                    ps = pst[:, 0:512]
                    for do in range(8):
                        nc.tensor.matmul(
                            ps, xtiles[sb][:, do, st * 128:(st + 1) * 128],
                            wv[:, do, :], start=(do == 0), stop=(do == 7))
                    nc.vector.tensor_copy(
                        va[:, sb * 4 + st, :, :DH],
                        ps.rearrange("p (h d) -> p h d", h=8))

            # ---- bf16 score matmul for head h, k-tile t, q-block qb ----
            def mm_score(out_ap, h, t, qb, c0, start, stop):
                g2, j = h % 2, h // 2
                p0 = 64 * g2
                lhsT = kt[p0:p0 + 64, j, 128 * t:128 * t + 128]
                rhs = qt[p0:p0 + 64, j, qb * 512 + c0:(qb + 1) * 512]
                nc.tensor.matmul(out_ap, lhsT, rhs, start=start, stop=stop)

            # ---- attention for (q-block qb, head h), generator-staged so
            # two heads' streams can be interleaved (widens the window for
            # hiding exp latency behind the other head's matmuls) ----
            def emit_av(ot, qb, h, et, u, nt):
                for half in range(2):
                    t = 2 * u + half
                    m = t - 4 * qb
                    c0 = 0 if m < 0 else 128 * m
                    hsl = 512 * half
                    nc.tensor.matmul(
                        ot[0:DH + 1, c0:512], va[:, t, h, :],
                        et[:, hsl + c0:hsl + 512],
                        start=(t == 0), stop=(t == nt - 1),
                        skip_group_check=True)

            def gen_c(qb, h):
                nt = 4 * qb + 4
                qsl = slice(qb * 512, (qb + 1) * 512)
                ot = ps_ot.tile([DH + 1, 512], F32, space="PSUM", tag="ot")
                prev = None  # (et, u) whose attn@V is deferred one round
                for u in range(nt // 2):
                    pt = ps_pair.tile([128, 1024], F32, space="PSUM", tag="pair")
                    et = ep.tile([128, 1024], BF16, tag="e")
                    masks = []
                    for half in range(2):
                        t = 2 * u + half
                        m = t - 4 * qb
                        hsl = 512 * half
                        if m < 0:  # full tile
                            mm_score(pt[:, hsl:hsl + 512], h, t, qb, 0,
                                     True, True)
                        else:      # diagonal tile: trim cols, mask strip later
                            c0 = 128 * m
                            mm_score(pt[:, hsl + c0:hsl + 512], h, t, qb, c0,
                                     True, False)
                            masks.append(hsl + c0)
                    # masks after both scores: their ldweights hide under the
                    # longer score streams
                    for o in masks:
                        nc.tensor.matmul(
                            pt[:, o:o + 128], negI[:], ltB[:],
                            start=False, stop=True, skip_group_check=True)
                    yield
                    # exp (ACT), batched over the pair when both halves full
                    m0 = 2 * u - 4 * qb
                    if m0 < 0:
                        nc.scalar.activation(et[:], pt[:], AF.Exp)
                    else:
                        c0a, c0b = 128 * m0, 128 * (m0 + 1)
                        nc.scalar.activation(
                            et[:, c0a:512], pt[:, c0a:512], AF.Exp)
                        nc.scalar.activation(
                            et[:, 512 + c0b:1024], pt[:, 512 + c0b:1024],
                            AF.Exp)
                    if prev is not None:
                        emit_av(ot, qb, h, prev[0], prev[1], nt)
                    prev = (et, u)
                    yield
                emit_av(ot, qb, h, prev[0], prev[1], nt)
                # softmax renorm: divide by ones-column row of ot
                dn = sp.tile([1, 512], F32, tag="dn")
                nc.vector.tensor_copy(dn[:], ot[DH:DH + 1, :])
                rec = sp.tile([1, 512], F32, tag="rec")
                nc.vector.reciprocal_approx_fast(rec[:], dn[:])
                rb = sp.tile([DH, 512], F32, tag="rb")
                nc.gpsimd.partition_broadcast(rb[:], rec[:])
                r0 = 64 * (h % 2)
                dst = on_[r0:r0 + 64, h // 2, qsl]
                nc.vector.tensor_tensor(dst, ot[0:DH, :], rb[:], ALU.mult)

            def emit_c_pair(qb, h0, h1):
                gens = [gen_c(qb, h0), gen_c(qb, h1)]
                alive = [True, True]
                while any(alive):
                    for i in (0, 1):
                        if alive[i]:
                            try:
                                next(gens[i])
                            except StopIteration:
                                alive[i] = False

            # ---- output projection for q-block qb: out^T[j, sq] ----
            def emit_proj(qb, jts):
                for jt in jts:
                    psjt = ps_pair.tile([128, 1024], F32, space="PSUM",
                                        tag="pair")
                    psj = psjt[:, 0:512]
                    for co in range(4):
                        nc.tensor.matmul(
                            psj, wp[:, co, jt * 128:(jt + 1) * 128],
                            on_[:, co, qb * 512:(qb + 1) * 512],
                            start=(co == 0), stop=(co == 3))
                    ot_sb = op.tile([128, 512], F32, tag="o")
                    nc.vector.tensor_scalar_add(ot_sb[:], psj,
                                                bps[:, jt:jt + 1])
                    nc.sync.dma_start(
                        out_d[jt * 128:(jt + 1) * 128,
                              qb * 512:(qb + 1) * 512],
                        ot_sb[:])

            # ---- emission: software-pipeline B(qb+1) and proj(qb-1) into
            # the attention loop over (qb, head-pair) ----
            for gi in range(12):
                emit_b_group(0, gi)
            for qb in range(4):
                if qb < 3:
                    emit_xdma(qb + 1)
                for hp in range(4):  # head pairs (2hp, 2hp+1)
                    if qb < 3:
                        for gi in range(3 * hp, 3 * hp + 3):
                            emit_b_group(qb + 1, gi)
                    emit_c_pair(qb, 2 * hp, 2 * hp + 1)
                    if qb >= 1:
                        emit_proj(qb - 1, [2 * hp, 2 * hp + 1])
            emit_proj(3, range(8))

    nc.compile()
    return nc


def kernel(x, Wq, bq, Wk, bk, Wv, bv, Wp, bp):
    global _NC
    if _NC is None:
        _NC = _build()

    x = np.asarray(x, np.float32)
    Wq, bq = np.asarray(Wq, np.float32), np.asarray(bq, np.float32)
    Wk, bk = np.asarray(Wk, np.float32), np.asarray(bk, np.float32)
    Wv, bv = np.asarray(Wv, np.float32), np.asarray(bv, np.float32)
    Wp, bp = np.asarray(Wp, np.float32), np.asarray(bp, np.float32)

    bf = ml_dtypes.bfloat16
    negI = np.ascontiguousarray((-30.0 * np.eye(128, dtype=np.float32)).astype(bf))
    i_ = np.arange(128)
    ltB = np.ascontiguousarray(
        (i_[None, :] < i_[:, None]).astype(np.float32).astype(bf))

    xbT = [np.ascontiguousarray(x[b].T.astype(bf)) for b in range(B)]

    in_maps = []
    for c in range(N_CORES):
        b, g = c // 2, c % 2
        hs = range(8 * g, 8 * g + 8)
        kidx = np.array([dh * 16 + h for h in hs for dh in range(DH)])
        fsl = slice(FG * g, FG * (g + 1))
        bp_c = (bp if g == 0 else 0.0) + Wp[:, fsl] @ bv[fsl]
        in_maps.append({
            "xbT": xbT[b],
            "wqT": np.ascontiguousarray((SCALE * Wq[fsl].T).astype(bf)),
            "wkT": np.ascontiguousarray(Wk[kidx].T.astype(bf)),
            "wvT": np.ascontiguousarray(Wv[fsl].T.astype(bf)),
            "wpT": np.ascontiguousarray(Wp[:, fsl].T.astype(bf)),
            "bqs": np.ascontiguousarray((SCALE * bq[fsl]).reshape(4, 128).T),
            "bks": np.ascontiguousarray(bk[kidx].reshape(4, 128).T),
            "bps": np.ascontiguousarray(bp_c.reshape(8, 128).T.astype(np.float32)),
            "negI": negI,
            "ltB": ltB,
        })

    res = run_bass_kernel_spmd(_NC, in_maps, core_ids=list(range(N_CORES)))
    out = np.empty((B, S, D), np.float32)
    for b in range(B):
        acc = res.results[2 * b]["outT"] + res.results[2 * b + 1]["outT"]
        out[b] = acc.T
    return out
